# revision 12
# baseline (speedup 1.0000x reference)
"""Multi-head attention (B=4, N=M=1024, D=KV=1024, H=16, HD=64) on 8 TRN2
NeuronCores, tensor-parallel over heads (2 heads per core).

Host: pre-transposes query/key_value to d-major, slices per-head weight
columns, converts the boolean mask to an additive bias; gathers per-core
attention blocks and partial output projections afterwards.

Device (per core, identical SPMD program):
  QT/KT = Wq/Wk^T @ xT   (combined 2-head projections, [128, n])
  V_nat = xT^T @ Wv      ([m, 128], with an appended ones column)
  scores^T[m, n] per head -> exp(scale*s + mask_bias) on ACT (mask fused,
  exp(-1e30) == 0) -> ctx matmul on unnormalized probs; the ones column
  yields softmax denominators in the same matmul.  A PE outer-product
  broadcasts 1/denom across partitions; DVE normalizes the stored
  attention.  Per-head [2, n] output projections accumulate on-chip.
"""
import os

import numpy as np

B, N, M, D = 4, 1024, 1024, 1024
H, HD = 16, 64
NCORES = 8
HL = H // NCORES  # heads per core
HW_ = HL * HD  # 128: local head width
DC = D // 128  # 8 contraction chunks
MC = M // 128  # 8 m chunks
NH = N // 512  # 2 moving-dim halves
SCALE = HD ** -0.5

# "bf16": cast inputs/stages to bfloat16 (fast).  "f32": full fp32 data with
# float32r matmuls (exact-ish).  Grading default is set at the bottom of the
# module after measurement.
_MODE = os.environ.get("MHA_MODE", "bf16")

_CACHE = {}


def _build(mode):
    import concourse.bacc as bacc
    import concourse.mybir as mybir
    import concourse.tile as tile

    dt = mybir.dt
    f32 = dt.float32
    # float32 matmul operands run 4x slower; float32r (same 4-byte fp32
    # bits, walrus rounds producer outputs) streams at full rate for
    # moving dims >= 256.  Tiles feeding matmuls are declared float32r.
    CDT = dt.bfloat16 if mode == "bf16" else dt.float32r
    ODT = CDT

    def mm(ap):
        return ap.bitcast(dt.float32r) if ap.dtype == f32 else ap

    nc = bacc.Bacc("TRN2", target_bir_lowering=False, debug=False)
    AF = mybir.ActivationFunctionType

    qT = nc.dram_tensor("qT", [B, D, N], CDT, kind="ExternalInput").ap()
    kvT = nc.dram_tensor("kvT", [B, D, M], CDT, kind="ExternalInput").ap()
    wq_d = nc.dram_tensor("wq", [D, HW_], CDT, kind="ExternalInput").ap()
    wk_d = nc.dram_tensor("wk", [D, HW_], CDT, kind="ExternalInput").ap()
    wv_d = nc.dram_tensor("wv", [D, HW_], CDT, kind="ExternalInput").ap()
    bq_d = nc.dram_tensor("bq", [1, HW_], CDT, kind="ExternalInput").ap()
    bk_d = nc.dram_tensor("bk", [1, HW_], CDT, kind="ExternalInput").ap()
    bv_d = nc.dram_tensor("bv", [1, HW_], CDT, kind="ExternalInput").ap()
    wp_d = nc.dram_tensor("wp", [HD, HL, 2], CDT, kind="ExternalInput").ap()
    mb_d = nc.dram_tensor("maskb", [B, 128, MC], f32, kind="ExternalInput").ap()
    ones_r_d = nc.dram_tensor("ones_r", [1, 512], CDT, kind="ExternalInput").ap()
    ones65_d = nc.dram_tensor(
        "ones65", [HD + 1, 128], dt.float32r, kind="ExternalInput"
    ).ap()
    onesp_d = nc.dram_tensor("onesp", [128, MC], CDT, kind="ExternalInput").ap()

    attn = nc.dram_tensor("attn", [B, HL, M, N], ODT, kind="ExternalOutput").ap()
    pout = nc.dram_tensor("pout", [B, 2, N], f32, kind="ExternalOutput").ap()

    with tile.TileContext(nc) as tc:
        with (
            tc.tile_pool(name="const", bufs=1) as const,
            tc.tile_pool(name="kvres", bufs=1) as kvres,
            tc.tile_pool(name="qin", bufs=3) as qin_p,
            tc.tile_pool(name="qk", bufs=2) as qk_p,
            tc.tile_pool(name="vn", bufs=2) as vn_p,
            tc.tile_pool(name="expp", bufs=2) as expp_p,
            tc.tile_pool(name="rbp", bufs=2) as rb_p,
            tc.tile_pool(name="pn", bufs=3) as pn_p,
            tc.tile_pool(name="ctxn", bufs=2) as ctxn_p,
            tc.tile_pool(name="sm", bufs=2) as sm_p,
            tc.tile_pool(name="psA", bufs=1, space="PSUM") as psA,  # proj/rb/po
            tc.tile_pool(name="psV", bufs=2, space="PSUM") as psV,
            tc.tile_pool(name="psS", bufs=2, space="PSUM") as psS,
            tc.tile_pool(name="psC", bufs=1, space="PSUM") as psC,
        ):
            # ---- constants ----
            wq_sb = const.tile([128, DC, HW_], CDT, name="wq_sb")
            nc.sync.dma_start(wq_sb[:], wq_d.rearrange("(c p) m -> p c m", p=128))
            wk_sb = const.tile([128, DC, HW_], CDT, name="wk_sb")
            nc.sync.dma_start(wk_sb[:], wk_d.rearrange("(c p) m -> p c m", p=128))
            wv_sb = const.tile([128, DC, HW_], CDT, name="wv_sb")
            nc.sync.dma_start(wv_sb[:], wv_d.rearrange("(c p) m -> p c m", p=128))
            bq_sb = const.tile([1, HW_], CDT, name="bq_sb")
            nc.sync.dma_start(bq_sb[:], bq_d)
            bk_sb = const.tile([1, HW_], CDT, name="bk_sb")
            nc.sync.dma_start(bk_sb[:], bk_d)
            bv_sb = const.tile([1, HW_], CDT, name="bv_sb")
            nc.sync.dma_start(bv_sb[:], bv_d)
            wp_sb = const.tile([HD, HL, 2], CDT, name="wp_sb")
            nc.sync.dma_start(wp_sb[:], wp_d)
            mb_sb = const.tile([128, B, MC], f32, name="mb_sb")
            nc.sync.dma_start(mb_sb[:], mb_d.rearrange("b p c -> p b c"))
            ones_r = const.tile([1, 512], CDT, name="ones_r")
            nc.sync.dma_start(ones_r[:], ones_r_d)
            ones65 = const.tile([HD + 1, 128], dt.float32r, name="ones65")
            nc.sync.dma_start(ones65[:], ones65_d)
            onesp = const.tile([128, MC], CDT, name="onesp")
            nc.sync.dma_start(onesp[:], onesp_d)

            for b in range(B):
                # ---- load kvT for this batch (resident) ----
                kv_t = kvres.tile([128, DC, M], CDT, tag="kv", name="kv_t")
                for dc in range(DC):
                    nc.sync.dma_start(
                        kv_t[:, dc, :], kvT[b, 128 * dc : 128 * (dc + 1), :]
                    )

                # ---- K^T projection (both heads): KT[128, m] ----
                kt_ps = psA.tile([128, M], f32, tag="A", name="kt_ps")
                for nh in range(NH):
                    sl = slice(512 * nh, 512 * (nh + 1))
                    for dc in range(DC):
                        nc.tensor.matmul(
                            kt_ps[:, sl],
                            mm(wk_sb[:, dc, :]),
                            mm(kv_t[:, dc, sl]),
                            start=(dc == 0),
                            stop=False,
                        )
                    nc.tensor.matmul(
                        kt_ps[:, sl],
                        mm(bk_sb[:]),
                        mm(ones_r[:]),
                        start=False,
                        stop=True,
                    )
                kt_sb = qk_p.tile([128, M], CDT, tag="kt", name="kt_sb")
                nc.vector.tensor_copy(kt_sb[:], kt_ps[:])

                # ---- V natural [m, 128] with ones columns ----
                vn_sb = vn_p.tile([128, MC, 2 * (HD + 1)], CDT, tag="vn", name="vn_sb")
                nc.vector.tensor_copy(vn_sb[:, :, HD : HD + 1], onesp[:].unsqueeze(2))
                nc.vector.tensor_copy(
                    vn_sb[:, :, 2 * HD + 1 : 2 * HD + 2], onesp[:].unsqueeze(2)
                )
                for mc in range(MC):
                    v_ps = psV.tile([128, HW_], f32, tag="V", name="v_ps")
                    for dc in range(DC):
                        nc.tensor.matmul(
                            v_ps[:],
                            mm(kv_t[:, dc, 128 * mc : 128 * (mc + 1)]),
                            mm(wv_sb[:, dc, :]),
                            start=(dc == 0),
                            stop=False,
                        )
                    nc.tensor.matmul(
                        v_ps[:],
                        mm(ones_r[:, 0:128]),
                        mm(bv_sb[:]),
                        start=False,
                        stop=True,
                    )
                    nc.vector.tensor_copy(vn_sb[:, mc, 0:HD], v_ps[:, 0:HD])
                    nc.vector.tensor_copy(
                        vn_sb[:, mc, HD + 1 : 2 * HD + 1], v_ps[:, HD : 2 * HD]
                    )

                # ---- Q^T projection (both heads): QT[128, n] ----
                qt_ps = psA.tile([128, N], f32, tag="A", name="qt_ps")
                q_ins = []
                for dc in range(DC):
                    q_in = qin_p.tile([128, N], CDT, tag="qin", name="q_in")
                    nc.sync.dma_start(q_in[:], qT[b, 128 * dc : 128 * (dc + 1), :])
                    q_ins.append(q_in)
                    for nh in range(NH):
                        sl = slice(512 * nh, 512 * (nh + 1))
                        nc.tensor.matmul(
                            qt_ps[:, sl],
                            mm(wq_sb[:, dc, :]),
                            mm(q_in[:, sl]),
                            start=(dc == 0),
                            stop=False,
                        )
                for nh in range(NH):
                    sl = slice(512 * nh, 512 * (nh + 1))
                    nc.tensor.matmul(
                        qt_ps[:, sl],
                        mm(bq_sb[:]),
                        mm(ones_r[:]),
                        start=False,
                        stop=True,
                    )
                qt_sb = qk_p.tile([128, N], CDT, tag="qt", name="qt_sb")
                nc.vector.tensor_copy(qt_sb[:], qt_ps[:])

                pout_sb = sm_p.tile([2, N], f32, tag="po_sb", name="pout_sb")

                for h in range(HL):
                    p0 = HD * h
                    ctx_ps = psC.tile([HD + 1, N], f32, tag="C", name="ctx_ps")
                    expp = expp_p.tile([128, MC, N], CDT, tag="expp", name="expp")
                    for mc in range(MC):
                        for nh in range(NH):
                            sl = slice(512 * nh, 512 * (nh + 1))
                            sc_ps = psS.tile([128, 512], f32, tag="S", name="sc_ps")
                            nc.tensor.matmul(
                                sc_ps[:],
                                mm(kt_sb[p0 : p0 + HD, 128 * mc : 128 * (mc + 1)]),
                                mm(qt_sb[p0 : p0 + HD, sl]),
                                start=True,
                                stop=True,
                            )
                            nc.scalar.activation(
                                expp[:, mc, sl],
                                sc_ps[:],
                                AF.Exp,
                                bias=mb_sb[:, b, mc : mc + 1],
                                scale=SCALE,
                            )
                            nc.tensor.matmul(
                                ctx_ps[:, sl],
                                mm(vn_sb[:, mc, (HD + 1) * h : (HD + 1) * (h + 1)]),
                                mm(expp[:, mc, sl]),
                                start=(mc == 0),
                                stop=(mc == MC - 1),
                            )

                    # softmax denominators live in ctx_ps row HD
                    rc = sm_p.tile([HD + 1, N], dt.float32r, tag="rc", name="rc")
                    with nc.allow_low_precision(reason="f32r is fp32-width"):
                        nc.vector.reciprocal(
                            rc[HD : HD + 1, :], ctx_ps[HD : HD + 1, :]
                        )
                    rb_ps = psA.tile([128, N], f32, tag="A", name="rb_ps")
                    for nh in range(NH):
                        sl = slice(512 * nh, 512 * (nh + 1))
                        nc.tensor.matmul(
                            rb_ps[:, sl],
                            mm(ones65[HD : HD + 1, :]),
                            mm(rc[HD : HD + 1, sl]),
                            start=True,
                            stop=True,
                        )
                    rb_sb = rb_p.tile([128, N], CDT, tag="rb", name="rb_sb")
                    nc.vector.tensor_copy(rb_sb[:], rb_ps[:])

                    for mc in range(MC):
                        pn = pn_p.tile([128, N], ODT, tag="pn", name="pn")
                        nc.vector.tensor_mul(pn[:], expp[:, mc, :], rb_sb[:])
                        nc.sync.dma_start(
                            attn[b, h, 128 * mc : 128 * (mc + 1), :], pn[:]
                        )

                    ctxn = ctxn_p.tile([HD, N], CDT, tag="ctxn", name="ctxn")
                    nc.vector.tensor_mul(ctxn[:], ctx_ps[0:HD, :], rb_sb[0:HD, :])

                    po_ps = psA.tile([2, N], f32, tag="A", name="po_ps")
                    for nh in range(NH):
                        sl = slice(512 * nh, 512 * (nh + 1))
                        nc.tensor.matmul(
                            po_ps[:, sl],
                            mm(wp_sb[:, h, :]),
                            mm(ctxn[:, sl]),
                            start=True,
                            stop=True,
                        )
                    if h == 0:
                        nc.vector.tensor_copy(pout_sb[:], po_ps[:])
                    else:
                        nc.vector.tensor_add(pout_sb[:], pout_sb[:], po_ps[:])
                nc.sync.dma_start(pout[b], pout_sb[:])

    nc.compile()
    return nc


def _get_nc(mode):
    if mode not in _CACHE:
        _CACHE[mode] = _build(mode)
    return _CACHE[mode]


def _np_dt(mode):
    if mode == "bf16":
        import ml_dtypes

        return np.dtype(ml_dtypes.bfloat16)
    return np.dtype(np.float32)


def _make_in_maps(query, key_value, mask, Wq, bq, Wkv, bkv, Wp, mode):
    cdt = _np_dt(mode)

    query = np.asarray(query, dtype=np.float32)
    key_value = np.asarray(key_value, dtype=np.float32)
    mask = np.asarray(mask)
    Wq = np.asarray(Wq, dtype=np.float32)
    bq = np.asarray(bq, dtype=np.float32)
    Wkv = np.asarray(Wkv, dtype=np.float32)
    bkv = np.asarray(bkv, dtype=np.float32)
    Wp = np.asarray(Wp, dtype=np.float32)

    qT = np.ascontiguousarray(query.transpose(0, 2, 1)).astype(cdt)
    kvT = np.ascontiguousarray(key_value.transpose(0, 2, 1)).astype(cdt)
    maskb = (
        np.where(mask, np.float32(-1e30), np.float32(0.0))
        .astype(np.float32)
        .reshape(B, MC, 128)
        .transpose(0, 2, 1)
        .copy()
    )

    in_maps = []
    for c in range(NCORES):
        lo = c * HW_
        in_maps.append(
            {
                "qT": qT,
                "kvT": kvT,
                "wq": Wq[:, lo : lo + HW_].astype(cdt),
                "wk": Wkv[:, lo : lo + HW_].astype(cdt),
                "wv": Wkv[:, D + lo : D + lo + HW_].astype(cdt),
                "bq": bq[lo : lo + HW_].reshape(1, HW_).astype(cdt),
                "bk": bkv[lo : lo + HW_].reshape(1, HW_).astype(cdt),
                "bv": bkv[D + lo : D + lo + HW_].reshape(1, HW_).astype(cdt),
                "wp": np.ascontiguousarray(
                    Wp[lo : lo + HW_, :].reshape(HL, HD, 2).transpose(1, 0, 2)
                ).astype(cdt),
                "maskb": maskb,
                "ones_r": np.ones((1, 512), dtype=cdt),
                "ones65": np.ones((HD + 1, 128), dtype=np.float32),
                "onesp": np.ones((128, MC), dtype=cdt),
            }
        )
    return in_maps


def kernel(query, key_value, mask, Wq, bq, Wkv, bkv, Wp, bp, _trace=False):
    from concourse import bass_utils

    mode = _MODE
    bp = np.asarray(bp, dtype=np.float32)
    in_maps = _make_in_maps(query, key_value, mask, Wq, bq, Wkv, bkv, Wp, mode)

    nc = _get_nc(mode)
    res = bass_utils.run_bass_kernel_spmd(
        nc, in_maps, core_ids=list(range(NCORES)), trace=_trace
    )

    # ---- gather ----
    # attn per core: [B, HL, M, N]; attention[b, n, m, 16]
    A = np.stack(
        [np.asarray(r["attn"], dtype=np.float32) for r in res.results], axis=0
    )  # [NC, B, HL, M, N]
    attention = np.ascontiguousarray(
        A.transpose(1, 4, 3, 0, 2).reshape(B, N, M, H)
    )
    po = sum(np.asarray(r["pout"], dtype=np.float32) for r in res.results)
    outputs = po.transpose(0, 2, 1) + bp[None, None, :]

    if _trace:
        kernel._last_results = res
    return outputs, attention


# revision 13
# speedup vs baseline: 1.0757x; 1.0757x over previous
"""Multi-head attention (B=4, N=M=1024, D=KV=1024, H=16, HD=64) on 8 TRN2
NeuronCores, tensor-parallel over heads (2 heads per core).

Host: pre-transposes query/key_value to d-major, slices per-head weight
columns, converts the boolean mask to an additive bias; gathers per-core
attention blocks and partial output projections afterwards.

Device (per core, identical SPMD program):
  QT/KT = Wq/Wk^T @ xT   (combined 2-head projections, [128, n])
  V_nat = xT^T @ Wv      ([m, 128], with an appended ones column)
  scores^T[m, n] per head -> exp(scale*s + mask_bias) on ACT (mask fused,
  exp(-1e30) == 0) -> ctx matmul on unnormalized probs; the ones column
  yields softmax denominators in the same matmul.  A PE outer-product
  broadcasts the denominator across partitions, one 128-lane DVE
  reciprocal turns it into 1/denom, and DVE normalizes the stored
  attention.  Per-head [2, n] output projections accumulate on-chip.
"""
import os

import numpy as np

B, N, M, D = 4, 1024, 1024, 1024
H, HD = 16, 64
NCORES = 8
HL = H // NCORES  # heads per core
HW_ = HL * HD  # 128: local head width
DC = D // 128  # 8 contraction chunks
MC = M // 128  # 8 m chunks
NH = N // 512  # 2 moving-dim halves
SCALE = HD ** -0.5

# "bf16": cast inputs/stages to bfloat16 (fast).  "f32": fp32 data with
# float32r matmuls.
_MODE = os.environ.get("MHA_MODE", "bf16")

_CACHE = {}


def _build(mode):
    import concourse.bacc as bacc
    import concourse.mybir as mybir
    import concourse.tile as tile

    dt = mybir.dt
    f32 = dt.float32
    # float32 matmul operands run 4x slower; float32r (same 4-byte fp32
    # bits, producers round) streams at full rate for moving dims >= 256.
    CDT = dt.bfloat16 if mode == "bf16" else dt.float32r
    ODT = CDT
    bf16 = mode == "bf16"

    def mm(ap):
        return ap.bitcast(dt.float32r) if ap.dtype == f32 else ap

    nc = bacc.Bacc("TRN2", target_bir_lowering=False, debug=False)
    AF = mybir.ActivationFunctionType

    qT = nc.dram_tensor("qT", [B, D, N], CDT, kind="ExternalInput").ap()
    kvT = nc.dram_tensor("kvT", [B, D, M], CDT, kind="ExternalInput").ap()
    wq_d = nc.dram_tensor("wq", [D, HW_], CDT, kind="ExternalInput").ap()
    wk_d = nc.dram_tensor("wk", [D, HW_], CDT, kind="ExternalInput").ap()
    wv_d = nc.dram_tensor("wv", [D, HW_], CDT, kind="ExternalInput").ap()
    bq_d = nc.dram_tensor("bq", [128, 1], f32, kind="ExternalInput").ap()
    bk_d = nc.dram_tensor("bk", [128, 1], f32, kind="ExternalInput").ap()
    bv_d = nc.dram_tensor("bv", [1, HW_], CDT, kind="ExternalInput").ap()
    wp_d = nc.dram_tensor("wp", [HD, HL, 2], CDT, kind="ExternalInput").ap()
    mb_d = nc.dram_tensor("maskb", [B, 128, MC], f32, kind="ExternalInput").ap()
    ones_r_d = nc.dram_tensor("ones_r", [1, 128], CDT, kind="ExternalInput").ap()
    ones65_d = nc.dram_tensor(
        "ones65", [HD + 1, 128], dt.float32r, kind="ExternalInput"
    ).ap()
    onesp_d = nc.dram_tensor("onesp", [128, MC], CDT, kind="ExternalInput").ap()

    attn = nc.dram_tensor("attn", [B, HL, M, N], ODT, kind="ExternalOutput").ap()
    pout = nc.dram_tensor("pout", [B, 2, N], f32, kind="ExternalOutput").ap()

    with tile.TileContext(nc) as tc:
        with (
            tc.tile_pool(name="const", bufs=1) as const,
            tc.tile_pool(name="kvres", bufs=1) as kvres,
            tc.tile_pool(name="qin", bufs=3 if bf16 else 2) as qin_p,
            tc.tile_pool(name="qk", bufs=2) as qk_p,
            tc.tile_pool(name="vn", bufs=2) as vn_p,
            tc.tile_pool(name="expp", bufs=2) as expp_p,
            tc.tile_pool(name="rbp", bufs=2) as rb_p,
            tc.tile_pool(name="pn", bufs=4 if bf16 else 2) as pn_p,
            tc.tile_pool(name="ctxn", bufs=2) as ctxn_p,
            tc.tile_pool(name="sm", bufs=2) as sm_p,
            # PSUM budget (8 banks): A 2x[128,512]=2, S 2x[128,512]=2,
            # R 1x[128,1024]=2, C 1x[65,1024]=2.
            tc.tile_pool(name="psA", bufs=2, space="PSUM") as psA,
            tc.tile_pool(name="psS", bufs=2, space="PSUM") as psS,
            tc.tile_pool(name="psR", bufs=1, space="PSUM") as psR,
            tc.tile_pool(name="psC", bufs=1, space="PSUM") as psC,
        ):
            # ---- constants ----
            wq_sb = const.tile([128, DC, HW_], CDT, name="wq_sb")
            nc.sync.dma_start(wq_sb[:], wq_d.rearrange("(c p) m -> p c m", p=128))
            wk_sb = const.tile([128, DC, HW_], CDT, name="wk_sb")
            nc.sync.dma_start(wk_sb[:], wk_d.rearrange("(c p) m -> p c m", p=128))
            wv_sb = const.tile([128, DC, HW_], CDT, name="wv_sb")
            nc.sync.dma_start(wv_sb[:], wv_d.rearrange("(c p) m -> p c m", p=128))
            bq_sb = const.tile([128, 1], f32, name="bq_sb")
            nc.sync.dma_start(bq_sb[:], bq_d)
            bk_sb = const.tile([128, 1], f32, name="bk_sb")
            nc.sync.dma_start(bk_sb[:], bk_d)
            bv_sb = const.tile([1, HW_], CDT, name="bv_sb")
            nc.sync.dma_start(bv_sb[:], bv_d)
            wp_sb = const.tile([HD, HL, 2], CDT, name="wp_sb")
            nc.sync.dma_start(wp_sb[:], wp_d)
            mb_sb = const.tile([128, B, MC], f32, name="mb_sb")
            nc.sync.dma_start(mb_sb[:], mb_d.rearrange("b p c -> p b c"))
            ones_r = const.tile([1, 128], CDT, name="ones_r")
            nc.sync.dma_start(ones_r[:], ones_r_d)
            ones65 = const.tile([HD + 1, 128], dt.float32r, name="ones65")
            nc.sync.dma_start(ones65[:], ones65_d)
            onesp = const.tile([128, MC], CDT, name="onesp")
            nc.sync.dma_start(onesp[:], onesp_d)

            for b in range(B):
                # ---- load kvT for this batch (resident) ----
                kv_t = kvres.tile([128, DC, M], CDT, tag="kv", name="kv_t")
                for dc in range(DC):
                    nc.sync.dma_start(
                        kv_t[:, dc, :], kvT[b, 128 * dc : 128 * (dc + 1), :]
                    )

                # ---- K^T projection (both heads): KT[128, m] ----
                kt_sb = qk_p.tile([128, M], CDT, tag="kt", name="kt_sb")
                for nh in range(NH):
                    sl = slice(512 * nh, 512 * (nh + 1))
                    kt_ps = psA.tile([128, 512], f32, tag="A", name="kt_ps")
                    for dc in range(DC):
                        nc.tensor.matmul(
                            kt_ps[:],
                            mm(wk_sb[:, dc, :]),
                            mm(kv_t[:, dc, sl]),
                            start=(dc == 0),
                            stop=(dc == DC - 1),
                        )
                    nc.vector.tensor_scalar_add(kt_sb[:, sl], kt_ps[:], bk_sb[:])

                # ---- Q^T projection (both heads): QT[128, n] ----
                qt_sb = qk_p.tile([128, N], CDT, tag="qt", name="qt_sb")
                q_ins = []
                for dc in range(DC):
                    q_in = qin_p.tile([128, N], CDT, tag="qin", name="q_in")
                    nc.sync.dma_start(q_in[:], qT[b, 128 * dc : 128 * (dc + 1), :])
                    q_ins.append(q_in)
                for nh in range(NH):
                    sl = slice(512 * nh, 512 * (nh + 1))
                    qt_ps = psA.tile([128, 512], f32, tag="A", name="qt_ps")
                    for dc in range(DC):
                        nc.tensor.matmul(
                            qt_ps[:],
                            mm(wq_sb[:, dc, :]),
                            mm(q_ins[dc][:, sl]),
                            start=(dc == 0),
                            stop=(dc == DC - 1),
                        )
                    nc.vector.tensor_scalar_add(qt_sb[:, sl], qt_ps[:], bq_sb[:])

                # ---- V natural [m, 128] with ones columns ----
                vn_sb = vn_p.tile([128, MC, 2 * (HD + 1)], CDT, tag="vn", name="vn_sb")
                nc.vector.tensor_copy(vn_sb[:, :, HD : HD + 1], onesp[:].unsqueeze(2))
                nc.vector.tensor_copy(
                    vn_sb[:, :, 2 * HD + 1 : 2 * HD + 2], onesp[:].unsqueeze(2)
                )
                for mc in range(MC):
                    v_ps = psA.tile([128, HW_], f32, tag="A", name="v_ps")
                    for dc in range(DC):
                        nc.tensor.matmul(
                            v_ps[:],
                            mm(kv_t[:, dc, 128 * mc : 128 * (mc + 1)]),
                            mm(wv_sb[:, dc, :]),
                            start=(dc == 0),
                            stop=False,
                        )
                    nc.tensor.matmul(
                        v_ps[:],
                        mm(ones_r[:]),
                        mm(bv_sb[:]),
                        start=False,
                        stop=True,
                    )
                    nc.vector.tensor_copy(vn_sb[:, mc, 0:HD], v_ps[:, 0:HD])
                    nc.vector.tensor_copy(
                        vn_sb[:, mc, HD + 1 : 2 * HD + 1], v_ps[:, HD : 2 * HD]
                    )

                pout_sb = sm_p.tile([2, N], f32, tag="po_sb", name="pout_sb")

                for h in range(HL):
                    p0 = HD * h
                    ctx_ps = psC.tile([HD + 1, N], f32, tag="C", name="ctx_ps")
                    expp = expp_p.tile([128, MC, N], CDT, tag="expp", name="expp")
                    for mc in range(MC):
                        for nh in range(NH):
                            sl = slice(512 * nh, 512 * (nh + 1))
                            sc_ps = psS.tile([128, 512], f32, tag="S", name="sc_ps")
                            nc.tensor.matmul(
                                sc_ps[:],
                                mm(kt_sb[p0 : p0 + HD, 128 * mc : 128 * (mc + 1)]),
                                mm(qt_sb[p0 : p0 + HD, sl]),
                                start=True,
                                stop=True,
                            )
                            nc.scalar.activation(
                                expp[:, mc, sl],
                                sc_ps[:],
                                AF.Exp,
                                bias=mb_sb[:, b, mc : mc + 1],
                                scale=SCALE,
                            )
                            nc.tensor.matmul(
                                ctx_ps[:, sl],
                                mm(vn_sb[:, mc, (HD + 1) * h : (HD + 1) * (h + 1)]),
                                mm(expp[:, mc, sl]),
                                start=(mc == 0),
                                stop=(mc == MC - 1),
                            )

                    # softmax denominators live in ctx_ps row HD: copy the
                    # row out, broadcast it over all partitions with a PE
                    # outer-product, then take a 128-lane reciprocal.
                    den = sm_p.tile([HD + 1, N], dt.float32r, tag="den", name="den")
                    nc.vector.tensor_copy(den[HD : HD + 1, :], ctx_ps[HD : HD + 1, :])
                    rb_ps = psR.tile([128, N], f32, tag="R", name="rb_ps")
                    for nh in range(NH):
                        sl = slice(512 * nh, 512 * (nh + 1))
                        nc.tensor.matmul(
                            rb_ps[:, sl],
                            mm(ones65[HD : HD + 1, :]),
                            mm(den[HD : HD + 1, sl]),
                            start=True,
                            stop=True,
                        )
                    rb_sb = rb_p.tile([128, N], CDT, tag="rb", name="rb_sb")
                    with nc.allow_low_precision(reason="recip of f32 psum"):
                        nc.vector.reciprocal(rb_sb[:], rb_ps[:])

                    for mc in range(MC):
                        pn = pn_p.tile([128, N], ODT, tag="pn", name="pn")
                        nc.vector.tensor_mul(pn[:], expp[:, mc, :], rb_sb[:])
                        nc.sync.dma_start(
                            attn[b, h, 128 * mc : 128 * (mc + 1), :], pn[:]
                        )

                    ctxn = ctxn_p.tile([HD, N], CDT, tag="ctxn", name="ctxn")
                    nc.vector.tensor_mul(ctxn[:], ctx_ps[0:HD, :], rb_sb[0:HD, :])

                    for nh in range(NH):
                        sl = slice(512 * nh, 512 * (nh + 1))
                        po_ps = psA.tile([2, 512], f32, tag="A", name="po_ps")
                        nc.tensor.matmul(
                            po_ps[:],
                            mm(wp_sb[:, h, :]),
                            mm(ctxn[:, sl]),
                            start=True,
                            stop=True,
                        )
                        if h == 0:
                            nc.vector.tensor_copy(pout_sb[:, sl], po_ps[:])
                        else:
                            nc.vector.tensor_add(
                                pout_sb[:, sl], pout_sb[:, sl], po_ps[:]
                            )
                nc.sync.dma_start(pout[b], pout_sb[:])

    nc.compile()
    return nc


def _get_nc(mode):
    if mode not in _CACHE:
        _CACHE[mode] = _build(mode)
    return _CACHE[mode]


def _np_dt(mode):
    if mode == "bf16":
        import ml_dtypes

        return np.dtype(ml_dtypes.bfloat16)
    return np.dtype(np.float32)


def _make_in_maps(query, key_value, mask, Wq, bq, Wkv, bkv, Wp, mode):
    cdt = _np_dt(mode)

    query = np.asarray(query, dtype=np.float32)
    key_value = np.asarray(key_value, dtype=np.float32)
    mask = np.asarray(mask)
    Wq = np.asarray(Wq, dtype=np.float32)
    bq = np.asarray(bq, dtype=np.float32)
    Wkv = np.asarray(Wkv, dtype=np.float32)
    bkv = np.asarray(bkv, dtype=np.float32)
    Wp = np.asarray(Wp, dtype=np.float32)

    qT = np.ascontiguousarray(query.transpose(0, 2, 1)).astype(cdt)
    kvT = np.ascontiguousarray(key_value.transpose(0, 2, 1)).astype(cdt)
    maskb = (
        np.where(mask, np.float32(-1e30), np.float32(0.0))
        .astype(np.float32)
        .reshape(B, MC, 128)
        .transpose(0, 2, 1)
        .copy()
    )

    in_maps = []
    for c in range(NCORES):
        lo = c * HW_
        in_maps.append(
            {
                "qT": qT,
                "kvT": kvT,
                "wq": Wq[:, lo : lo + HW_].astype(cdt),
                "wk": Wkv[:, lo : lo + HW_].astype(cdt),
                "wv": Wkv[:, D + lo : D + lo + HW_].astype(cdt),
                "bq": bq[lo : lo + HW_].reshape(128, 1).copy(),
                "bk": bkv[lo : lo + HW_].reshape(128, 1).copy(),
                "bv": bkv[D + lo : D + lo + HW_].reshape(1, HW_).astype(cdt),
                "wp": np.ascontiguousarray(
                    Wp[lo : lo + HW_, :].reshape(HL, HD, 2).transpose(1, 0, 2)
                ).astype(cdt),
                "maskb": maskb,
                "ones_r": np.ones((1, 128), dtype=cdt),
                "ones65": np.ones((HD + 1, 128), dtype=np.float32),
                "onesp": np.ones((128, MC), dtype=cdt),
            }
        )
    return in_maps


def kernel(query, key_value, mask, Wq, bq, Wkv, bkv, Wp, bp, _trace=False):
    from concourse import bass_utils

    mode = _MODE
    bp = np.asarray(bp, dtype=np.float32)
    in_maps = _make_in_maps(query, key_value, mask, Wq, bq, Wkv, bkv, Wp, mode)

    nc = _get_nc(mode)
    res = bass_utils.run_bass_kernel_spmd(
        nc, in_maps, core_ids=list(range(NCORES)), trace=_trace
    )

    # ---- gather ----
    # attn per core: [B, HL, M, N]; attention[b, n, m, 16]
    A = np.stack(
        [np.asarray(r["attn"], dtype=np.float32) for r in res.results], axis=0
    )  # [NC, B, HL, M, N]
    attention = np.ascontiguousarray(
        A.transpose(1, 4, 3, 0, 2).reshape(B, N, M, H)
    )
    po = sum(np.asarray(r["pout"], dtype=np.float32) for r in res.results)
    outputs = po.transpose(0, 2, 1) + bp[None, None, :]

    if _trace:
        kernel._last_results = res
    return outputs, attention
